# revision 2
# baseline (speedup 1.0000x reference)
"""TRN2 Bass kernel for GQA MultiHeadAttention (B=2, S=2048, D=2048, 16 q-heads,
4 kv-heads, d_k=128) with QK-RMSNorm + interleaved RoPE + causal softmax + out-proj.

Sharding: 8 cores = (batch b in {0,1}) x (kv-head group g in {0..3}).
Each core computes its 4 q-heads' attention for its batch and a partial
out-projection y.T = Wo_g @ attn_out_g.T  [2048(e) x 2048(s)].
Host sums the 4 partials per batch and transposes.

Device layouts (all "head-dim on partitions", so no on-device transposes):
  xT   [d=16x128, s]        (moving operand of all projections)
  qT/kT [c=128, s]          RoPE'd + normalized, bf16
  V    [s-in-block=128, 16 blocks, c=128]
  scores ST [j=128, i<=512] via matmul(lhsT=kT-block, rhs=qT-tile)
  P = exp(ST * c0 * rrk[j]) masked; AO.T [c, i] = sum_j V.T P
No softmax max-subtraction: RMSNorm bounds |score| <= sqrt(128), exp is safe.
RoPE pair-interleave is folded into a host-side row permutation of Wq/Wk
(dot products are invariant; V/Wo untouched).
"""
import sys
import numpy as np
import ml_dtypes

sys.path.insert(0, "/opt/trn_rl_repo")

import concourse.bass as bass  # noqa: E402
import concourse.tile as tile  # noqa: E402
from concourse import mybir  # noqa: E402
from concourse.bass_utils import run_bass_kernel_spmd  # noqa: E402


def _ensure_ntff_hook():
    """bass_utils' trace path imports antenv.axon_hooks; some images lack it.
    Register an equivalent shim (same ctypes hook trn_boot would install)."""
    try:
        import antenv.axon_hooks  # noqa: F401
        return
    except ImportError:
        pass
    import types
    try:
        import antenv
        from trn_agent_boot.trn_boot import _ntff_profile_via_ctypes
        hook = [_ntff_profile_via_ctypes("/opt/axon/libaxon_pjrt.so")]
    except Exception:
        return
    mod = types.ModuleType("antenv.axon_hooks")
    mod.get_axon_ntff_profile_hook = lambda: hook[0]
    mod.set_axon_ntff_profile_hook = lambda h: hook.__setitem__(0, h)
    sys.modules["antenv.axon_hooks"] = mod
    antenv.axon_hooks = mod


_ensure_ntff_hook()

F32 = mybir.dt.float32
F32R = mybir.dt.float32r
BF16 = mybir.dt.bfloat16
AF = mybir.ActivationFunctionType

P = 128
S = 2048
D = 2048
DK = 128
NH_LOC = 4          # q heads per core
NC_CHUNKS = D // P  # 16 contraction chunks
N_STILE = 4         # s-tiles of 512
STILE = 512
NJB = S // P        # 16 j/s blocks of 128
EPS = 1e-8
C0 = 1.0 / np.sqrt(DK)

_BF = ml_dtypes.bfloat16


_NO_SPLIT_OPCODES = {"UnconditionalBranch", "Call", "RegisterMove", "EventSemaphore"}
_WAIT_LIMIT = {}  # hw instruction structs take a single sync wait


def _split_excess_waits(nc):
    """Walrus codegen allows only 1-2 sync waits per instruction struct; Tile
    can emit more. Move excess waits onto same-engine NoOps inserted before."""
    import bass_rust
    counter = [0]
    for fn in nc.m.functions:
        for blk in fn.blocks:
            out = []
            changed = False
            for inst in blk.instructions:
                si = inst.sync_info
                limit = _WAIT_LIMIT.get(inst.opcode, 1)
                if (si is not None and len(si.on_wait) > limit
                        and inst.opcode not in _NO_SPLIT_OPCODES):
                    waits = list(si.on_wait)
                    for w in waits[:-limit]:
                        counter[0] += 1
                        nop = bass_rust.InstNoOp(
                            name=f"I-wsplit-{counter[0]}", ins=[], outs=[])
                        nop.engine = inst.engine
                        nop.sync_info = mybir.SyncInfo(on_wait=[w], on_update=[])
                        out.append(nop)
                    inst.sync_info = mybir.SyncInfo(
                        on_wait=waits[-limit:], on_update=list(si.on_update))
                    changed = True
                out.append(inst)
            if changed:
                blk.instructions = out
    return counter[0]


def _build_program():
    nc = bass.Bass()

    ext = {}

    def inp(name, shape, dt):
        ext[name] = nc.declare_dram_parameter(name, list(shape), dt, isOutput=False)
        return ext[name]

    xt4 = inp("xt4", (N_STILE, NC_CHUNKS, P, STILE), BF16)
    wq = inp("wq", (NC_CHUNKS, P, NH_LOC * DK), BF16)
    wk = inp("wk", (NC_CHUNKS, P, DK), BF16)
    wv = inp("wv", (NC_CHUNKS, P, DK), BF16)
    wo = inp("wo", (NH_LOC, P, D), BF16)
    winvq = inp("winvq", (P, 1), BF16)
    winvk = inp("winvk", (P, 1), BF16)
    taba = inp("taba", (P, S), F32)   # [cos; cos]
    tabb = inp("tabb", (P, S), F32)   # [-sin; sin]
    maskt = inp("maskt", (P, NH_LOC, STILE), BF16)
    bq = inp("bq", (P, NH_LOC), F32)
    bk = inp("bk", (P, 1), F32)
    boeff = inp("boeff", (P, NJB), F32)
    onesr = inp("onesr", (1, P), F32R)
    yT = nc.declare_dram_parameter("yT", [D, S], F32, isOutput=True)

    from contextlib import ExitStack

    with tile.TileContext(nc) as tc, ExitStack() as top:
        const = top.enter_context(tc.tile_pool(name="const", bufs=1))

        wq_sb = const.tile([P, NC_CHUNKS, NH_LOC * DK], BF16, tag="wq")
        wk_sb = const.tile([P, NC_CHUNKS, DK], BF16, tag="wk")
        wv_sb = const.tile([P, NC_CHUNKS, DK], BF16, tag="wv")
        winvq_sb = const.tile([P, 1], BF16, tag="winvq")
        winvk_sb = const.tile([P, 1], BF16, tag="winvk")
        taba_sb = const.tile([P, S], F32, tag="taba")
        tabb_sb = const.tile([P, S], F32, tag="tabb")
        mask_sb = const.tile([P, NH_LOC, STILE], BF16, tag="mask")
        bq_sb = const.tile([P, NH_LOC], F32, tag="bq")
        bk_sb = const.tile([P, 1], F32, tag="bk")
        bo_sb = const.tile([P, NJB], F32, tag="bo")
        wo_sb = const.tile([P, NH_LOC, D], BF16, tag="wo")

        ones1_f = const.tile([1, P], F32R, tag="ones1")
        nc.sync.dma_start(ones1_f[:], onesr[:])
        onesc_b = const.tile([P, 1], BF16, tag="onesc")
        nc.vector.memset(onesc_b[:], 1.0)
        eps_sb = const.tile([P, 1], F32, tag="eps")
        nc.vector.memset(eps_sb[:], EPS)

        # persistent activation tensors
        qhat = const.tile([P, NH_LOC, S], BF16, tag="qhat")   # [c, h, s]
        khat = const.tile([P, S], BF16, tag="khat")           # [c, s]
        vsb = const.tile([P, NJB, DK], BF16, tag="v")         # [s%128, block, c]
        aon = const.tile([P, NH_LOC, S], BF16, tag="aon")     # [c, h, i]
        scale_k = const.tile([P, NJB], F32, tag="scale_k")    # c0 * rrk, [j%128, block]

        # ------- Phase 1: projections + RMS + RoPE, fused per (output, s-tile) -------
        with ExitStack() as ph1:
            xp = ph1.enter_context(tc.tile_pool(name="xp", bufs=2))
            t1p = ph1.enter_context(tc.tile_pool(name="t1p", bufs=4))
            rp = ph1.enter_context(tc.tile_pool(name="rp", bufs=3))
            ps1 = ph1.enter_context(tc.tile_pool(name="ps1", bufs=3, space="PSUM"))
            psl = ph1.enter_context(tc.tile_pool(name="psl", bufs=1, space="PSUM"))
            pskl = ph1.enter_context(tc.tile_pool(name="pskl", bufs=1, space="PSUM"))
            psb = ph1.enter_context(tc.tile_pool(name="psb", bufs=2, space="PSUM"))

            ps_kl = pskl.tile([P, NJB], F32, tag="pskl")

            # startup-critical loads first: q weights + first x tile, interleaved
            xt0 = xp.tile([P, NC_CHUNKS, STILE], BF16, tag="xt", name="xt0")
            nc.sync.dma_start(wq_sb[:, 0:1, :], wq[0:1].rearrange("c p m -> p c m"))
            nc.sync.dma_start(xt0[:, 0:1, :], xt4[0, 0:1].rearrange("c p s -> p c s"))
            for g4 in range(0, NC_CHUNKS, 4):
                sl4 = slice(max(g4, 1), g4 + 4)
                nc.sync.dma_start(wq_sb[:, sl4, :], wq[sl4].rearrange("c p m -> p c m"))
                nc.sync.dma_start(xt0[:, sl4, :],
                                  xt4[0, sl4].rearrange("c p s -> p c s"))
            nc.sync.dma_start(bq_sb[:], bq[:])
            nc.sync.dma_start(winvq_sb[:], winvq[:])
            nc.sync.dma_start(taba_sb[:], taba[:])
            nc.sync.dma_start(tabb_sb[:], tabb[:])
            for g4 in range(0, NC_CHUNKS, 4):
                sl4 = slice(g4, g4 + 4)
                nc.sync.dma_start(wk_sb[:, sl4, :], wk[sl4].rearrange("c p m -> p c m"))
                nc.sync.dma_start(wv_sb[:, sl4, :], wv[sl4].rearrange("c p m -> p c m"))
            nc.sync.dma_start(winvk_sb[:], winvk[:])
            nc.sync.dma_start(bk_sb[:], bk[:])

            for st in range(N_STILE):
                if st == 0:
                    xt = xt0
                else:
                    xt = xp.tile([P, NC_CHUNKS, STILE], BF16, tag="xt")
                    for g4 in range(0, NC_CHUNKS, 4):
                        sl4 = slice(g4, g4 + 4)
                        nc.sync.dma_start(xt[:, sl4, :],
                                          xt4[st, sl4].rearrange("c p s -> p c s"))
                ssl = bass.ts(st, STILE)

                # q heads first, then v, then k (k last: its psum banks are
                # freed by the fast ACT reader right before phase 2 reuses them)
                for oi in list(range(NH_LOC)) + ["v", "k"]:
                    if oi == "v":
                        # v: output [s-block=128, c=128], 4 s-blocks per s-tile
                        ptv = ps1.tile([P, STILE], F32, tag="proj", name="ptv")
                        for sb in range(4):
                            for ch in range(NC_CHUNKS):
                                nc.tensor.matmul(ptv[:, bass.ts(sb, DK)],
                                                 xt[:, ch, bass.ts(sb, P)], wv_sb[:, ch, :],
                                                 start=(ch == 0), stop=(ch == NC_CHUNKS - 1))
                        for sb in range(4):
                            nc.vector.tensor_copy(vsb[:, st * 4 + sb, :], ptv[:, bass.ts(sb, DK)])
                        continue
                    is_q = oi != "k"
                    pt = ps1.tile([P, STILE], F32, tag="proj")
                    for ch in range(NC_CHUNKS):
                        lw = wq_sb[:, ch, bass.ts(oi, DK)] if is_q else wk_sb[:, ch, :]
                        nc.tensor.matmul(pt[:], lw, xt[:, ch, :],
                                         start=(ch == 0), stop=(ch == NC_CHUNKS - 1))
                    bias_ap = bq_sb[:, oi : oi + 1] if is_q else bk_sb[:, 0:1]
                    qf = t1p.tile([P, STILE], F32, tag="qf")
                    nc.scalar.activation(qf[:], pt[:], AF.Identity, bias=bias_ap)

                    # sumsq for rms (q: [1,s] orientation; k: transposed [s,1])
                    sq = t1p.tile([P, STILE], BF16, tag="sq")
                    nc.vector.tensor_mul(sq[:], qf[:], qf[:])
                    if is_q:
                        pl = psl.tile([1, STILE], F32, tag="psl")
                        nc.tensor.matmul(pl[:], winvq_sb[:], sq[:], start=True, stop=True)
                        sq_f = t1p.tile([1, STILE], F32, tag="sqf")
                        nc.scalar.activation(sq_f[:], pl[:], AF.Sqrt,
                                             bias=eps_sb[0:1, :], scale=1.0 / DK)
                        rrq_t = t1p.tile([1, STILE], F32R, tag="rrq")
                        with nc.allow_low_precision(reason="f32r (11-bit mantissa) broadcast"):
                            nc.vector.reciprocal(rrq_t[:], sq_f[:])
                        pb = psb.tile([P, STILE], F32, tag="rqb")
                        nc.tensor.matmul(pb[:], ones1_f[:], rrq_t[:],
                                         start=True, stop=True)
                    else:
                        for sb in range(4):
                            nc.tensor.matmul(ps_kl[:, st * 4 + sb : st * 4 + sb + 1],
                                             sq[:, bass.ts(sb, P)], winvk_sb[:],
                                             start=True, stop=True)

                    # RoPE on de-interleaved halves:
                    #   rt = qf*[cos;cos] + swap_halves(qf)*[-sin;sin]
                    sw = rp.tile([P, STILE], F32, tag="sw")
                    nc.sync.dma_start(sw[0:64, :], qf[64:P, :])
                    nc.sync.dma_start(sw[64:P, :], qf[0:64, :])
                    ta = rp.tile([P, STILE], F32, tag="ta")
                    tb = rp.tile([P, STILE], F32, tag="tb")
                    nc.vector.tensor_mul(ta[:], qf[:], taba_sb[:, ssl])
                    nc.vector.tensor_mul(tb[:], sw[:], tabb_sb[:, ssl])
                    if is_q:
                        rt = rp.tile([P, STILE], F32, tag="rope")
                        nc.vector.tensor_add(rt[:], ta[:], tb[:])
                        nc.vector.tensor_mul(qhat[:, oi, ssl], rt[:], pb[:])
                    else:
                        nc.vector.tensor_add(khat[:, ssl], ta[:], tb[:])

                # k-side scale for this s-tile: c0 / rms_k as [j%128, block]
                ksl4 = bass.ts(st, 4)
                skf = t1p.tile([P, 4], F32, tag="skf")
                nc.scalar.activation(skf[:], ps_kl[:, ksl4], AF.Sqrt,
                                     bias=eps_sb[:], scale=1.0 / DK)
                nc.vector.reciprocal(skf[:], skf[:])
                nc.vector.tensor_scalar_mul(scale_k[:, ksl4], skf[:], C0)




        # ---------------- Phase 2: attention ----------------
        nc.sync.dma_start(mask_sb[:], maskt[:])
        nc.sync.dma_start(wo_sb[:], wo.rearrange("f p e -> p f e"))
        nc.sync.dma_start(bo_sb[:], boeff[:])

        with ExitStack() as ph2:
            pp = ph2.enter_context(tc.tile_pool(name="pp", bufs=4))
            lp = ph2.enter_context(tc.tile_pool(name="lp", bufs=2))
            rlp = ph2.enter_context(tc.tile_pool(name="rlp", bufs=2))
            psst = ph2.enter_context(tc.tile_pool(name="psst", bufs=3, space="PSUM"))
            psao = ph2.enter_context(tc.tile_pool(name="psao", bufs=2, space="PSUM"))
            psli = ph2.enter_context(tc.tile_pool(name="psli", bufs=2, space="PSUM"))
            psrl = ph2.enter_context(tc.tile_pool(name="psrl", bufs=1, space="PSUM"))

            for h in range(NH_LOC):
                for it in range(N_STILE):
                    isl = bass.ts(it, STILE)
                    njb = 4 * it + 4
                    ao_ps = psao.tile([P, STILE], F32, tag="ao")
                    pl = psli.tile([1, STILE], F32, tag="li")
                    # ascending jb: group head (jb=0) is always full-width, so
                    # later diagonal blocks can write narrowed column ranges.
                    for idx, jb in enumerate(range(njb)):
                        t = jb - 4 * it
                        lo = P * t if t > 0 else 0  # masked-out prefix columns
                        csl = slice(lo, STILE)
                        i0 = it * STILE + lo
                        st_ps = psst.tile([P, STILE], F32, tag="st")
                        nc.tensor.matmul(st_ps[:, csl], khat[:, bass.ts(jb, P)],
                                         qhat[:, h, bass.ds(i0, STILE - lo)],
                                         start=True, stop=True)
                        pt = pp.tile([P, STILE], BF16, tag="p")
                        nc.scalar.activation(pt[:, csl], st_ps[:, csl], AF.Exp,
                                             scale=scale_k[:, jb : jb + 1])
                        if t >= 0:
                            nc.vector.tensor_mul(pt[:, csl], pt[:, csl],
                                                 mask_sb[:, t, csl])
                        nc.tensor.matmul(ao_ps[:, csl], vsb[:, jb, :], pt[:, csl],
                                         start=(idx == 0), stop=(idx == njb - 1))
                        # softmax denominator rides PE: l += ones.T @ P
                        nc.tensor.matmul(pl[:, csl], onesc_b[:], pt[:, csl],
                                         start=(idx == 0), stop=(idx == njb - 1))
                    rl = lp.tile([1, STILE], F32R, tag="rl")
                    with nc.allow_low_precision(reason="f32r (11-bit mantissa) broadcast"):
                        nc.vector.reciprocal(rl[:], pl[:])
                    rlb = psrl.tile([P, STILE], F32, tag="rlb")
                    nc.tensor.matmul(rlb[:], ones1_f[:], rl[:], start=True, stop=True)
                    rlb_sb = rlp.tile([P, STILE], F32, tag="rlbs")
                    nc.vector.tensor_copy(rlb_sb[:], rlb[:])
                    nc.vector.tensor_mul(aon[:, h, isl], ao_ps[:], rlb_sb[:])

        # ---------------- Phase 3: out-projection ----------------
        with ExitStack() as ph3:
            yp = ph3.enter_context(tc.tile_pool(name="yp", bufs=3))
            psy = ph3.enter_context(tc.tile_pool(name="psy", bufs=6, space="PSUM"))

            yT_v = yT.rearrange("(eb p) s -> eb p s", p=P)
            for eb in range(NJB):
                y_sb = yp.tile([P, S], F32, tag="ysb")
                for st in range(N_STILE):
                    yps = psy.tile([P, STILE], F32, tag="y")
                    for fc in range(NH_LOC):
                        nc.tensor.matmul(yps[:], wo_sb[:, fc, bass.ts(eb, P)],
                                         aon[:, fc, bass.ts(st, STILE)],
                                         start=(fc == 0), stop=(fc == NH_LOC - 1))
                    nc.vector.tensor_scalar_add(y_sb[:, bass.ts(st, STILE)], yps[:],
                                                bo_sb[:, eb : eb + 1])
                nc.sync.dma_start(yT_v[eb], y_sb[:])

    _split_excess_waits(nc)
    return nc


_PERM = np.concatenate([np.arange(0, DK, 2), np.arange(1, DK, 2)])  # de-interleave


def _prep_inputs(x, Wq, bq, Wk, bk, Wv, bv, Wo, bo, q_norm_w, k_norm_w):
    """Build the 8 per-core input maps. Core c -> (b = c // 4, g = c % 4)."""
    def bf(a):
        return np.ascontiguousarray(a).astype(_BF)

    wq_p = q_norm_w[_PERM].astype(np.float32)
    wk_p = k_norm_w[_PERM].astype(np.float32)
    with np.errstate(divide="ignore"):
        winvq = np.where(wq_p != 0, 1.0 / np.maximum(wq_p * wq_p, 1e-30), 0.0)
        winvk = np.where(wk_p != 0, 1.0 / np.maximum(wk_p * wk_p, 1e-30), 0.0)

    inv_freq = 1.0 / (10000.0 ** (np.arange(0, DK, 2, dtype=np.float32) / np.float32(DK)))
    freqs = np.arange(S, dtype=np.float32)[:, None] * inv_freq[None, :]
    cosT = np.cos(freqs).T.astype(np.float32)  # [64, S]
    sinT = np.sin(freqs).T.astype(np.float32)
    taba = np.vstack([cosT, cosT]).copy()          # [128, S]
    tabb = np.vstack([-sinT, sinT]).copy()

    pj = np.arange(P)[:, None, None]
    tt = np.arange(NH_LOC)[None, :, None]
    fi = np.arange(STILE)[None, None, :]
    maskt = ((P * tt + pj) <= fi).astype(np.float32)  # [128, 4, 512]

    xt4_b = []
    for b in range(2):
        xt = x[b].T.astype(np.float32)  # [d, s]
        xt4_b.append(bf(xt.reshape(NC_CHUNKS, P, N_STILE, STILE).transpose(2, 0, 1, 3)))

    in_maps = []
    for core in range(8):
        b, g = divmod(core, NH_LOC)
        hsl = slice(g * NH_LOC * DK, (g + 1) * NH_LOC * DK)
        ksl = slice(g * DK, (g + 1) * DK)

        wq_blk = Wq[hsl].astype(np.float32).copy()  # [512, d]
        # per-head de-interleave permutation + fold q_norm_w
        wq_blk = wq_blk.reshape(NH_LOC, DK, D)[:, _PERM, :] * wq_p[None, :, None]
        wq_t = wq_blk.reshape(NH_LOC * DK, D).T.reshape(NC_CHUNKS, P, NH_LOC * DK)

        wk_blk = Wk[ksl].astype(np.float32)[_PERM, :] * wk_p[:, None]
        wk_t = wk_blk.T.reshape(NC_CHUNKS, P, DK)
        wv_t = Wv[ksl].astype(np.float32).T.reshape(NC_CHUNKS, P, DK)
        wo_t = Wo[:, hsl].astype(np.float32).T.reshape(NH_LOC, P, D)

        bq_blk = bq[hsl].astype(np.float32).reshape(NH_LOC, DK)[:, _PERM].T.copy()  # [128, 4]
        bk_blk = bk[ksl].astype(np.float32)[_PERM][:, None].copy()
        if g == 0:
            boeff = bo.astype(np.float32).reshape(NJB, P).T.copy()
        else:
            boeff = np.zeros((P, NJB), np.float32)

        in_maps.append({
            "xt4": xt4_b[b],
            "wq": bf(wq_t), "wk": bf(wk_t), "wv": bf(wv_t), "wo": bf(wo_t),
            "winvq": bf(winvq[:, None]), "winvk": bf(winvk[:, None]),
            "taba": taba, "tabb": tabb,
            "maskt": bf(maskt),
            "bq": np.ascontiguousarray(bq_blk), "bk": bk_blk, "boeff": boeff,
            "onesr": np.ones((1, P), np.float32),
        })
    return in_maps


_CACHED = {}


def _get_program():
    if "nc" not in _CACHED:
        _CACHED["nc"] = _build_program()
    return _CACHED["nc"]


def kernel(x, Wq, bq, Wk, bk, Wv, bv, Wo, bo, q_norm_w, k_norm_w, _trace=False, _tmpdir=None):
    x = np.asarray(x, np.float32)
    args = [np.asarray(a, np.float32) for a in
            (Wq, bq, Wk, bk, Wv, bv, Wo, bo, q_norm_w, k_norm_w)]
    Wq, bq, Wk, bk, Wv, bv, Wo, bo, q_norm_w, k_norm_w = args

    nc = _get_program()
    in_maps = _prep_inputs(x, Wq, bq, Wk, bk, Wv, bv, Wo, bo, q_norm_w, k_norm_w)
    res = run_bass_kernel_spmd(nc, in_maps, list(range(8)), trace=_trace, tmpdir=_tmpdir)

    out = np.zeros((2, S, D), np.float32)
    for core in range(8):
        b = core // 4
        out[b] += res.results[core]["yT"].T
    # v-bias enters only via softmax-weighted average (weights sum to 1):
    if np.any(bv):
        out += (np.repeat(bv.reshape(4, DK), 4, axis=0).reshape(D) @ Wo.T)[None, None, :]
    kernel._last_result = res
    return out



# revision 10
# speedup vs baseline: 1.0540x; 1.0540x over previous
"""TRN2 Bass kernel for GQA MultiHeadAttention (B=2, S=2048, D=2048, 16 q-heads,
4 kv-heads, d_k=128) with QK-RMSNorm + interleaved RoPE + causal softmax + out-proj.

Sharding: 8 cores = (batch b in {0,1}) x (kv-head group g in {0..3}).
Each core computes its 4 q-heads' attention for its batch and a partial
out-projection y.T = Wo_g @ attn_out_g.T  [2048(e) x 2048(s)] in fp16.
Host sums the 4 partials per batch and transposes.

Single software-pipelined loop over s-tiles st=0..3:
    proj(st) -> attn(it=st-1) -> outproj(st-2)
so every dependent PE op (sumsq matmuls, rms/softmax-normalize broadcasts)
is emitted behind >=1 section of independent matmul work and the PE never
drains (keeps the HAM clock gate at K=8/8).

Device layouts (all "head-dim on partitions", no on-device transposes):
  xT   [d=16x128, s]        (moving operand of all projections)
  qhat/khat [c=128, s]      RoPE'd (+q-normalized), bf16
  V    [s-in-block=128, 16 blocks, c=128]
  scores ST [j=128, i<=512] via matmul(lhsT=khat-block, rhs=qhat-tile)
  P = exp(ST * scale_k[j]) masked; AO.T [c, i] = sum_j V.T P
  softmax denominator rides PE (ones.T @ P); 1/l via fast-approx DVE recip.
No softmax max-subtraction: RMSNorm bounds |score| <= sqrt(128), exp is safe.
RoPE pair-interleave is folded into a host-side row permutation of Wq/Wk.
"""
import sys
import numpy as np
import ml_dtypes

sys.path.insert(0, "/opt/trn_rl_repo")

import concourse.bass as bass  # noqa: E402
import concourse.tile as tile  # noqa: E402
from concourse import mybir  # noqa: E402
from concourse.bass_utils import run_bass_kernel_spmd  # noqa: E402


def _ensure_ntff_hook():
    """bass_utils' trace path imports antenv.axon_hooks; some images lack it.
    Register an equivalent shim (same ctypes hook trn_boot would install)."""
    try:
        import antenv.axon_hooks  # noqa: F401
        return
    except ImportError:
        pass
    import types
    try:
        import antenv
        from trn_agent_boot.trn_boot import _ntff_profile_via_ctypes
        hook = [_ntff_profile_via_ctypes("/opt/axon/libaxon_pjrt.so")]
    except Exception:
        return
    mod = types.ModuleType("antenv.axon_hooks")
    mod.get_axon_ntff_profile_hook = lambda: hook[0]
    mod.set_axon_ntff_profile_hook = lambda h: hook.__setitem__(0, h)
    sys.modules["antenv.axon_hooks"] = mod
    antenv.axon_hooks = mod


_ensure_ntff_hook()

F32 = mybir.dt.float32
BF16 = mybir.dt.bfloat16
F16 = mybir.dt.float16
AF = mybir.ActivationFunctionType

P = 128
S = 2048
D = 2048
DK = 128
NH_LOC = 4          # q heads per core
NC_CHUNKS = D // P  # 16 contraction chunks
N_STILE = 4         # s-tiles of 512
STILE = 512
NJB = S // P        # 16 j/s blocks of 128
EPS = 1e-8
C0 = 1.0 / np.sqrt(DK)
N_WARM = 8          # PE warm-up matmuls (HAM clock-gate)

_BF = ml_dtypes.bfloat16


_NO_SPLIT_OPCODES = {"UnconditionalBranch", "Call", "RegisterMove", "EventSemaphore"}
_WAIT_LIMIT = {}  # hw instruction structs take a single sync wait


def _split_excess_waits(nc):
    """Walrus codegen allows only 1-2 sync waits per instruction struct; Tile
    can emit more. Move excess waits onto same-engine NoOps inserted before."""
    import bass_rust
    counter = [0]
    for fn in nc.m.functions:
        for blk in fn.blocks:
            out = []
            changed = False
            for inst in blk.instructions:
                si = inst.sync_info
                limit = _WAIT_LIMIT.get(inst.opcode, 1)
                if (si is not None and len(si.on_wait) > limit
                        and inst.opcode not in _NO_SPLIT_OPCODES):
                    waits = list(si.on_wait)
                    for w in waits[:-limit]:
                        counter[0] += 1
                        nop = bass_rust.InstNoOp(
                            name=f"I-wsplit-{counter[0]}", ins=[], outs=[])
                        nop.engine = inst.engine
                        nop.sync_info = mybir.SyncInfo(on_wait=[w], on_update=[])
                        out.append(nop)
                    inst.sync_info = mybir.SyncInfo(
                        on_wait=waits[-limit:], on_update=list(si.on_update))
                    changed = True
                out.append(inst)
            if changed:
                blk.instructions = out
    return counter[0]


def _build_program():
    nc = bass.Bass()

    ext = {}

    def inp(name, shape, dt):
        ext[name] = nc.declare_dram_parameter(name, list(shape), dt, isOutput=False)
        return ext[name]

    xt4 = inp("xt4", (N_STILE, NC_CHUNKS, P, STILE), BF16)
    wq = inp("wq", (NC_CHUNKS, P, NH_LOC * DK), BF16)
    wk = inp("wk", (NC_CHUNKS, P, DK), BF16)
    wv = inp("wv", (NC_CHUNKS, P, DK), BF16)
    wo = inp("wo", (NH_LOC, P, D), BF16)
    winvq = inp("winvq", (P, 1), BF16)   # 1/(w_q^2 * DK)
    winvk = inp("winvk", (P, 1), BF16)   # 1/w_k^2
    taba = inp("taba", (P, S), BF16)     # [cos; cos]
    tabb = inp("tabb", (P, S), BF16)     # [-sin; sin]
    maskt = inp("maskt", (P, NH_LOC, STILE), BF16)
    bq = inp("bq", (P, NH_LOC), F32)
    bk = inp("bk", (P, 1), F32)
    boeff = inp("boeff", (P, NJB), F32)
    yT = nc.declare_dram_parameter("yT", [D, S], F16, isOutput=True)

    from contextlib import ExitStack

    with tile.TileContext(nc) as tc, ExitStack() as top:
        const = top.enter_context(tc.tile_pool(name="const", bufs=1))

        wq_sb = const.tile([P, NC_CHUNKS, NH_LOC * DK], BF16, tag="wq")
        wk_sb = const.tile([P, NC_CHUNKS, DK], BF16, tag="wk")
        wv_sb = const.tile([P, NC_CHUNKS, DK], BF16, tag="wv")
        wo_sb = const.tile([P, NH_LOC, D], BF16, tag="wo")
        winvq_sb = const.tile([P, 1], BF16, tag="winvq")
        winvk_sb = const.tile([P, 1], BF16, tag="winvk")
        taba_sb = const.tile([P, S], BF16, tag="taba")
        tabb_sb = const.tile([P, S], BF16, tag="tabb")
        mask_sb = const.tile([P, NH_LOC, STILE], BF16, tag="mask")
        bq_sb = const.tile([P, NH_LOC], F32, tag="bq")
        bk_sb = const.tile([P, 1], F32, tag="bk")
        bo_sb = const.tile([P, NJB], F32, tag="bo")

        onesc_b = const.tile([P, 1], BF16, tag="onesc")
        nc.vector.memset(onesc_b[:], 1.0)
        onesr_b = const.tile([1, P], BF16, tag="onesr")
        nc.vector.memset(onesr_b[:], 1.0)
        wrow = const.tile([1, STILE], BF16, tag="wrow")
        nc.vector.memset(wrow[:], 1.0)
        eps_sb = const.tile([P, 1], F32, tag="eps")
        nc.vector.memset(eps_sb[:], EPS)
        epsk_sb = const.tile([P, 1], F32, tag="epsk")
        nc.vector.memset(epsk_sb[:], DK * EPS)

        # persistent activation tensors
        khat = const.tile([P, S], BF16, tag="khat")           # [c, s]
        vsb = const.tile([P, NJB, DK], BF16, tag="v")         # [s%128, block, c]
        scale_k = const.tile([P, NJB], F32, tag="scale_k")    # c0/rms_k, [j%128, block]

        # rotating pools
        xp = top.enter_context(tc.tile_pool(name="xp", bufs=2))
        qhp = top.enter_context(tc.tile_pool(name="qhp", bufs=2))
        aop = top.enter_context(tc.tile_pool(name="aop", bufs=2))
        qfp = top.enter_context(tc.tile_pool(name="qfp", bufs=3))
        sqp = top.enter_context(tc.tile_pool(name="sqp", bufs=3))
        swp = top.enter_context(tc.tile_pool(name="swp", bufs=3))
        t2p = top.enter_context(tc.tile_pool(name="t2p", bufs=4))
        rtp = top.enter_context(tc.tile_pool(name="rtp", bufs=3))
        rowp = top.enter_context(tc.tile_pool(name="rowp", bufs=4))
        rowbp = top.enter_context(tc.tile_pool(name="rowbp", bufs=4))
        pp = top.enter_context(tc.tile_pool(name="pp", bufs=4))
        rlbp = top.enter_context(tc.tile_pool(name="rlbp", bufs=2))
        yp = top.enter_context(tc.tile_pool(name="yp", bufs=3))

        rot = top.enter_context(tc.tile_pool(name="rot", bufs=3, space="PSUM"))
        aops = top.enter_context(tc.tile_pool(name="aops", bufs=2, space="PSUM"))
        plq = top.enter_context(tc.tile_pool(name="plq", bufs=1, space="PSUM"))
        pl2 = top.enter_context(tc.tile_pool(name="pl2", bufs=1, space="PSUM"))
        pskl = top.enter_context(tc.tile_pool(name="pskl", bufs=1, space="PSUM"))

        plq_bank = plq.tile([P, STILE], F32, tag="plq")
        pl2_bank = pl2.tile([P, STILE], F32, tag="pl2")
        ps_kl = pskl.tile([P, NJB], F32, tag="pskl")

        # -------- deferred-emission queue (PE ops needing pipeline cover) ----
        pending = []

        def flush_pending():
            while pending:
                pending.pop(0)()

        # -------------------- startup DMAs --------------------
        # order: what proj(k, st=0) needs first, then v, then q, then attn/out.
        xt0 = xp.tile([P, NC_CHUNKS, STILE], BF16, tag="xt", name="xt0")
        nc.sync.dma_start(xt0[:, 0:4, :], xt4[0, 0:4].rearrange("c p s -> p c s"))
        for g4 in range(0, NC_CHUNKS, 4):
            sl4 = slice(g4, g4 + 4)
            nc.sync.dma_start(wk_sb[:, sl4, :], wk[sl4].rearrange("c p m -> p c m"))
        nc.sync.dma_start(bk_sb[:], bk[:])
        nc.sync.dma_start(winvk_sb[:], winvk[:])
        for g4 in range(0, NC_CHUNKS, 4):
            sl4 = slice(g4, g4 + 4)
            nc.sync.dma_start(wv_sb[:, sl4, :], wv[sl4].rearrange("c p m -> p c m"))
        nc.sync.dma_start(taba_sb[:, 0:STILE], taba[:, 0:STILE])
        nc.sync.dma_start(tabb_sb[:, 0:STILE], tabb[:, 0:STILE])
        for g4 in range(4, NC_CHUNKS, 4):
            sl4 = slice(g4, g4 + 4)
            nc.sync.dma_start(xt0[:, sl4, :], xt4[0, sl4].rearrange("c p s -> p c s"))
        for g4 in range(0, NC_CHUNKS, 4):
            sl4 = slice(g4, g4 + 4)
            nc.sync.dma_start(wq_sb[:, sl4, :], wq[sl4].rearrange("c p m -> p c m"))
        nc.sync.dma_start(bq_sb[:], bq[:])
        nc.sync.dma_start(winvq_sb[:], winvq[:])
        for st_ in range(1, N_STILE):
            ssl_ = bass.ts(st_, STILE)
            nc.sync.dma_start(taba_sb[:, ssl_], taba[:, ssl_])
            nc.sync.dma_start(tabb_sb[:, ssl_], tabb[:, ssl_])

        # -------------------- PE warm-up (HAM clock gate) --------------------
        for w_ in range(N_WARM):
            wt = rot.tile([P, STILE], F32, tag="rot", name=f"warm{w_}")
            nc.tensor.matmul(wt[:], onesr_b[:], wrow[:], start=True, stop=True)

        # state carried across pipeline stages
        qhat_t = {}   # st -> tile [P, NH_LOC, STILE]
        aon_t = {}    # st -> tile [P, NH_LOC, STILE]
        xt_t = {0: xt0}

        # ==================== section emitters ====================

        def emit_proj_mm(st, oi, xt):
            """PE projection matmuls for one output (q-head / 'k' / 'v')."""
            if oi == "v":
                ptv = rot.tile([P, STILE], F32, tag="rot", name=f"ptv{st}")
                for sb in range(4):
                    for ch in range(NC_CHUNKS):
                        nc.tensor.matmul(ptv[:, bass.ts(sb, DK)],
                                         xt[:, ch, bass.ts(sb, P)], wv_sb[:, ch, :],
                                         start=(ch == 0), stop=(ch == NC_CHUNKS - 1))
                return ptv
            pt = rot.tile([P, STILE], F32, tag="rot", name=f"pt{st}{oi}")
            for ch in range(NC_CHUNKS):
                lw = wk_sb[:, ch, :] if oi == "k" else wq_sb[:, ch, bass.ts(oi, DK)]
                nc.tensor.matmul(pt[:], lw, xt[:, ch, :],
                                 start=(ch == 0), stop=(ch == NC_CHUNKS - 1))
            return pt

        def emit_tail1(st, oi, pt):
            """Off-PE tail right after proj matmuls: bias+bf16 (ACT), sumsq,
            RoPE (DVE). Returns handles used by the deferred PE tail."""
            ssl = bass.ts(st, STILE)
            if oi == "v":
                nc.vector.tensor_copy(vsb[:, st * 4:(st + 1) * 4, :], pt[:])
                return None
            is_q = oi != "k"
            bias_ap = bq_sb[:, oi:oi + 1] if is_q else bk_sb[:, 0:1]
            qf = qfp.tile([P, STILE], BF16, tag="qf")
            nc.scalar.activation(qf[:], pt[:], AF.Identity, bias=bias_ap)
            sq = sqp.tile([P, STILE], BF16, tag="sq")
            nc.vector.tensor_mul(sq[:], qf[:], qf[:])
            # RoPE on de-interleaved halves:
            #   rt = qf*[cos;cos] + swap_halves(qf)*[-sin;sin]
            sw = swp.tile([P, STILE], BF16, tag="sw")
            nc.sync.dma_start(sw[0:64, :], qf[64:P, :])
            nc.sync.dma_start(sw[64:P, :], qf[0:64, :])
            ta = t2p.tile([P, STILE], BF16, tag="ta")
            tb = t2p.tile([P, STILE], BF16, tag="tb")
            nc.vector.tensor_mul(ta[:], qf[:], taba_sb[:, ssl])
            nc.vector.tensor_mul(tb[:], sw[:], tabb_sb[:, ssl])
            if is_q:
                rt = rtp.tile([P, STILE], BF16, tag="rt")
                nc.vector.tensor_add(rt[:], ta[:], tb[:])
                return sq, rt
            nc.vector.tensor_add(khat[:, ssl], ta[:], tb[:])
            return sq, None

        def emit_pe_tail1(st, oi, sq):
            """PE sumsq for output oi (emitted one proj-group later)."""
            if oi == "k":
                for sb in range(4):
                    col = st * 4 + sb
                    nc.tensor.matmul(ps_kl[:, col:col + 1],
                                     sq[:, bass.ts(sb, P)], winvk_sb[:],
                                     start=True, stop=True)
                # scale_k = C0/rms_k = 1/sqrt(DK*ms + DK*eps) = exp(-ln(.)/2)
                ksl4 = bass.ts(st, 4)
                skf = rowp.tile([P, 4], F32, tag="skf")
                nc.scalar.activation(skf[:], ps_kl[:, ksl4], AF.Ln,
                                     bias=epsk_sb[:], scale=1.0)
                nc.scalar.activation(scale_k[:, ksl4], skf[:], AF.Exp, scale=-0.5)
            else:
                h = oi
                out = plq_bank[32 * h:32 * h + 1, :]
                nc.tensor.matmul(out, winvq_sb[:], sq[:], start=True, stop=True,
                                 tile_position=(0, 32 * h))
                # rrq = 1/rms_q = 1/sqrt(ms + eps) = exp(-ln(ms + eps)/2)
                rrq_f = rowp.tile([1, STILE], F32, tag="rrqf")
                nc.scalar.activation(rrq_f[:], plq_bank[32 * h:32 * h + 1, :],
                                     AF.Ln, bias=eps_sb[0:1, :], scale=1.0)
                rrq_b = rowbp.tile([1, STILE], BF16, tag="rrqb")
                nc.scalar.activation(rrq_b[:], rrq_f[:], AF.Exp, scale=-0.5)
                return rrq_b

        def emit_pe_tail2(st, h, rt, rrq_b):
            """PE broadcast of 1/rms_q + final qhat mul (two groups later)."""
            pb = rot.tile([P, STILE], F32, tag="rot", name=f"pb{st}{h}")
            nc.tensor.matmul(pb[:], onesr_b[:], rrq_b[:], start=True, stop=True)
            nc.vector.tensor_mul(qhat_t[st][:, h, :], rt[:], pb[:])

        def emit_proj(st):
            xt = xt_t[st]
            qhat_t[st] = qhp.tile([P, NH_LOC, STILE], BF16, tag="qhat", name=f"qhat{st}")

            seq = ["k", "v", 0, 1, 2, 3]
            tails = {}
            for idx, oi in enumerate(seq):
                pt = emit_proj_mm(st, oi, xt)
                if idx >= 1:
                    flush_pending()
                tails[oi] = emit_tail1(st, oi, pt)
                prev = seq[idx - 1] if idx >= 1 else None
                if prev is not None and tails[prev] is not None:
                    sq_p, rt_p = tails[prev]
                    rrq_b = emit_pe_tail1(st, prev, sq_p)
                    if rt_p is not None:
                        pending.append(
                            lambda st=st, h=prev, rt=rt_p, rb=rrq_b:
                            emit_pe_tail2(st, h, rt, rb))
            # tail of the last output (q3): sumsq now, broadcast deferred
            sq_p, rt_p = tails[3]
            rrq_b = emit_pe_tail1(st, 3, sq_p)
            pending.append(
                lambda st=st, rt=rt_p, rb=rrq_b: emit_pe_tail2(st, 3, rt, rb))
            # prefetch next x tile; late-stage weights after the first prefetch
            if st + 1 < N_STILE:
                xt_n = xp.tile([P, NC_CHUNKS, STILE], BF16, tag="xt",
                               name=f"xt{st + 1}")
                for g4 in range(0, NC_CHUNKS, 4):
                    sl4 = slice(g4, g4 + 4)
                    nc.sync.dma_start(xt_n[:, sl4, :],
                                      xt4[st + 1, sl4].rearrange("c p s -> p c s"))
                xt_t[st + 1] = xt_n
            if st == 0:
                nc.sync.dma_start(mask_sb[:], maskt[:])
                nc.sync.dma_start(bo_sb[:], boeff[:])
            elif st == 1:
                nc.sync.dma_start(wo_sb[:], wo.rearrange("f p e -> p f e"))

        def emit_attn_tile_end(st, h, ao_ps):
            """softmax-normalize: 1/l, broadcast on PE, multiply into aon."""
            rl_f = rowp.tile([1, STILE], F32, tag="rlf")
            nc.scalar.activation(rl_f[:], pl2_bank[32 * h:32 * h + 1, :], AF.Ln)
            rl_b = rowbp.tile([1, STILE], BF16, tag="rlb")
            nc.scalar.activation(rl_b[:], rl_f[:], AF.Exp, scale=-1.0)

            def bcast(st=st, h=h, ao_ps=ao_ps, rl_b=rl_b):
                rlb = rot.tile([P, STILE], F32, tag="rot", name=f"rlb{st}{h}")
                nc.tensor.matmul(rlb[:], onesr_b[:], rl_b[:], start=True, stop=True)
                rlb_sb = rlbp.tile([P, STILE], BF16, tag="rlbs")
                nc.vector.tensor_copy(rlb_sb[:], rlb[:])
                nc.vector.tensor_mul(aon_t[st][:, h, :], ao_ps[:], rlb_sb[:])
            pending.append(bcast)

        def emit_attn(it):
            aon_t[it] = aop.tile([P, NH_LOC, STILE], BF16, tag="aon", name=f"aon{it}")
            njb = 4 * it + 4
            SKEW = 2
            for h in range(NH_LOC):
                ao_ps = aops.tile([P, STILE], F32, tag="ao")
                pts = {}
                colsl = {}
                for x in range(njb + SKEW):
                    if x < njb:
                        jb = x
                        t = jb - 4 * it
                        lo = P * t if t > 0 else 0
                        csl = slice(lo, STILE)
                        colsl[jb] = (csl, t)
                        i0 = it * STILE + lo
                        st_ps = rot.tile([P, STILE], F32, tag="rot",
                                         name=f"st{it}{h}{jb}")
                        nc.tensor.matmul(st_ps[:, csl], khat[:, bass.ts(jb, P)],
                                         qhat_t[it][:, h, bass.ds(lo, STILE - lo)],
                                         start=True, stop=True)
                        pt8 = pp.tile([P, STILE], BF16, tag="p")
                        nc.scalar.activation(pt8[:, csl], st_ps[:, csl], AF.Exp,
                                             scale=scale_k[:, jb:jb + 1])
                        if t >= 0:
                            nc.vector.tensor_mul(pt8[:, csl], pt8[:, csl],
                                                 mask_sb[:, t, csl])
                        pts[jb] = pt8
                        if x == 1:
                            flush_pending()
                    if x >= SKEW:
                        jb = x - SKEW
                        csl, t = colsl[jb]
                        pt8 = pts.pop(jb)
                        nc.tensor.matmul(ao_ps[:, csl], vsb[:, jb, :], pt8[:, csl],
                                         start=(jb == 0), stop=(jb == njb - 1))
                        nc.tensor.matmul(pl2_bank[32 * h:32 * h + 1, csl],
                                         onesc_b[:], pt8[:, csl],
                                         start=(jb == 0), stop=(jb == njb - 1),
                                         tile_position=(0, 32 * h))
                emit_attn_tile_end(it, h, ao_ps)

        def emit_outproj(st):
            yT_v = yT.rearrange("(eb p) s -> eb p s", p=P)
            for eb in range(NJB):
                yps = rot.tile([P, STILE], F32, tag="rot", name=f"y{st}{eb}")
                for fc in range(NH_LOC):
                    nc.tensor.matmul(yps[:], wo_sb[:, fc, bass.ts(eb, P)],
                                     aon_t[st][:, fc, :],
                                     start=(fc == 0), stop=(fc == NH_LOC - 1))
                if eb % 4 == 1:
                    flush_pending()
                y_sb = yp.tile([P, STILE], F16, tag="ysb")
                nc.vector.tensor_scalar_add(y_sb[:], yps[:], bo_sb[:, eb:eb + 1])
                nc.sync.dma_start(yT_v[eb, :, bass.ts(st, STILE)], y_sb[:])

        # ==================== main pipelined loop ====================
        for st in range(N_STILE):
            emit_proj(st)
            if st >= 1:
                emit_attn(st - 1)
            if st >= 2:
                emit_outproj(st - 2)
        emit_attn(3)
        emit_outproj(2)
        emit_outproj(3)
        flush_pending()

    _split_excess_waits(nc)
    return nc


_PERM = np.concatenate([np.arange(0, DK, 2), np.arange(1, DK, 2)])  # de-interleave


def _prep_inputs(x, Wq, bq, Wk, bk, Wv, bv, Wo, bo, q_norm_w, k_norm_w):
    """Build the 8 per-core input maps. Core c -> (b = c // 4, g = c % 4)."""
    def bf(a):
        return np.ascontiguousarray(a).astype(_BF)

    wq_p = q_norm_w[_PERM].astype(np.float32)
    wk_p = k_norm_w[_PERM].astype(np.float32)
    with np.errstate(divide="ignore"):
        winvq = np.where(wq_p != 0,
                         1.0 / np.maximum(wq_p * wq_p * DK, 1e-30), 0.0)
        winvk = np.where(wk_p != 0, 1.0 / np.maximum(wk_p * wk_p, 1e-30), 0.0)

    inv_freq = 1.0 / (10000.0 ** (np.arange(0, DK, 2, dtype=np.float32) / np.float32(DK)))
    freqs = np.arange(S, dtype=np.float32)[:, None] * inv_freq[None, :]
    cosT = np.cos(freqs).T.astype(np.float32)  # [64, S]
    sinT = np.sin(freqs).T.astype(np.float32)
    taba = np.vstack([cosT, cosT]).copy()          # [128, S]
    tabb = np.vstack([-sinT, sinT]).copy()

    pj = np.arange(P)[:, None, None]
    tt = np.arange(NH_LOC)[None, :, None]
    fi = np.arange(STILE)[None, None, :]
    maskt = ((P * tt + pj) <= fi).astype(np.float32)  # [128, 4, 512]

    xt4_b = []
    for b in range(2):
        xt = x[b].T.astype(np.float32)  # [d, s]
        xt4_b.append(bf(xt.reshape(NC_CHUNKS, P, N_STILE, STILE).transpose(2, 0, 1, 3)))

    in_maps = []
    for core in range(8):
        b, g = divmod(core, NH_LOC)
        hsl = slice(g * NH_LOC * DK, (g + 1) * NH_LOC * DK)
        ksl = slice(g * DK, (g + 1) * DK)

        wq_blk = Wq[hsl].astype(np.float32).copy()  # [512, d]
        # per-head de-interleave permutation + fold q_norm_w
        wq_blk = wq_blk.reshape(NH_LOC, DK, D)[:, _PERM, :] * wq_p[None, :, None]
        wq_t = wq_blk.reshape(NH_LOC * DK, D).T.reshape(NC_CHUNKS, P, NH_LOC * DK)

        wk_blk = Wk[ksl].astype(np.float32)[_PERM, :] * wk_p[:, None]
        wk_t = wk_blk.T.reshape(NC_CHUNKS, P, DK)
        wv_t = Wv[ksl].astype(np.float32).T.reshape(NC_CHUNKS, P, DK)
        wo_t = Wo[:, hsl].astype(np.float32).T.reshape(NH_LOC, P, D)

        # fold the norm weight into the bias too (wq has w folded in)
        bq_blk = (bq[hsl].astype(np.float32).reshape(NH_LOC, DK)[:, _PERM]
                  * wq_p[None, :]).T.copy()  # [128, 4]
        bk_blk = (bk[ksl].astype(np.float32)[_PERM] * wk_p)[:, None].copy()
        if g == 0:
            boeff = bo.astype(np.float32).reshape(NJB, P).T.copy()
        else:
            boeff = np.zeros((P, NJB), np.float32)

        in_maps.append({
            "xt4": xt4_b[b],
            "wq": bf(wq_t), "wk": bf(wk_t), "wv": bf(wv_t), "wo": bf(wo_t),
            "winvq": bf(winvq[:, None]), "winvk": bf(winvk[:, None]),
            "taba": bf(taba), "tabb": bf(tabb),
            "maskt": bf(maskt),
            "bq": np.ascontiguousarray(bq_blk), "bk": bk_blk, "boeff": boeff,
        })
    return in_maps


_CACHED = {}


def _get_program():
    if "nc" not in _CACHED:
        _CACHED["nc"] = _build_program()
    return _CACHED["nc"]


def kernel(x, Wq, bq, Wk, bk, Wv, bv, Wo, bo, q_norm_w, k_norm_w, _trace=False, _tmpdir=None):
    x = np.asarray(x, np.float32)
    args = [np.asarray(a, np.float32) for a in
            (Wq, bq, Wk, bk, Wv, bv, Wo, bo, q_norm_w, k_norm_w)]
    Wq, bq, Wk, bk, Wv, bv, Wo, bo, q_norm_w, k_norm_w = args

    nc = _get_program()
    in_maps = _prep_inputs(x, Wq, bq, Wk, bk, Wv, bv, Wo, bo, q_norm_w, k_norm_w)
    res = run_bass_kernel_spmd(nc, in_maps, list(range(8)), trace=_trace, tmpdir=_tmpdir)

    out = np.zeros((2, S, D), np.float32)
    for core in range(8):
        b = core // 4
        out[b] += res.results[core]["yT"].astype(np.float32).T
    # v-bias enters only via softmax-weighted average (weights sum to 1):
    if np.any(bv):
        out += (np.repeat(bv.reshape(4, DK), 4, axis=0).reshape(D) @ Wo.T)[None, None, :]
    kernel._last_result = res
    return out


# revision 11
# speedup vs baseline: 1.2403x; 1.1767x over previous
"""TRN2 Bass kernel for GQA MultiHeadAttention (B=2, S=2048, D=2048, 16 q-heads,
4 kv-heads, d_k=128) with QK-RMSNorm + interleaved RoPE + causal softmax + out-proj.

Sharding: 8 cores = (batch b in {0,1}) x (kv-head group g in {0..3}).
Each core computes its 4 q-heads' attention for its batch and a partial
out-projection y.T = Wo_g @ attn_out_g.T  [2048(e) x 2048(s)] in fp16.
Host sums the 4 partials per batch and transposes.

Single software-pipelined loop over s-tiles st=0..3:
    proj(st) -> attn(it=st-1) -> outproj(st-2)
so every dependent PE op (sumsq matmuls, rms/softmax-normalize broadcasts)
is emitted behind >=1 section of independent matmul work and the PE never
drains (keeps the HAM clock gate at K=8/8).

Device layouts (all "head-dim on partitions", no on-device transposes):
  xT   [d=16x128, s]        (moving operand of all projections)
  qhat/khat [c=128, s]      RoPE'd (+q-normalized), bf16
  V    [s-in-block=128, 16 blocks, c=128]
  scores ST [j=128, i<=512] via matmul(lhsT=khat-block, rhs=qhat-tile)
  P = exp(ST * scale_k[j]) masked; AO.T [c, i] = sum_j V.T P
  softmax denominator rides PE (ones.T @ P); 1/l via fast-approx DVE recip.
No softmax max-subtraction: RMSNorm bounds |score| <= sqrt(128), exp is safe.
RoPE pair-interleave is folded into a host-side row permutation of Wq/Wk.
"""
import sys
import numpy as np
import ml_dtypes

sys.path.insert(0, "/opt/trn_rl_repo")

import concourse.bass as bass  # noqa: E402
import concourse.tile as tile  # noqa: E402
from concourse import mybir  # noqa: E402
from concourse.bass_utils import run_bass_kernel_spmd  # noqa: E402


def _ensure_ntff_hook():
    """bass_utils' trace path imports antenv.axon_hooks; some images lack it.
    Register an equivalent shim (same ctypes hook trn_boot would install)."""
    try:
        import antenv.axon_hooks  # noqa: F401
        return
    except ImportError:
        pass
    import types
    try:
        import antenv
        from trn_agent_boot.trn_boot import _ntff_profile_via_ctypes
        hook = [_ntff_profile_via_ctypes("/opt/axon/libaxon_pjrt.so")]
    except Exception:
        return
    mod = types.ModuleType("antenv.axon_hooks")
    mod.get_axon_ntff_profile_hook = lambda: hook[0]
    mod.set_axon_ntff_profile_hook = lambda h: hook.__setitem__(0, h)
    sys.modules["antenv.axon_hooks"] = mod
    antenv.axon_hooks = mod


_ensure_ntff_hook()

F32 = mybir.dt.float32
BF16 = mybir.dt.bfloat16
F16 = mybir.dt.float16
AF = mybir.ActivationFunctionType

P = 128
S = 2048
D = 2048
DK = 128
NH_LOC = 4          # q heads per core
NC_CHUNKS = D // P  # 16 contraction chunks
N_STILE = 4         # s-tiles of 512
STILE = 512
NJB = S // P        # 16 j/s blocks of 128
EPS = 1e-8
C0 = 1.0 / np.sqrt(DK)
N_WARM = 8          # PE warm-up matmuls (HAM clock-gate)

_BF = ml_dtypes.bfloat16


_NO_SPLIT_OPCODES = {"UnconditionalBranch", "Call", "RegisterMove", "EventSemaphore"}
_WAIT_LIMIT = {}  # hw instruction structs take a single sync wait


def _split_excess_waits(nc):
    """Walrus codegen allows only 1-2 sync waits per instruction struct; Tile
    can emit more. Move excess waits onto same-engine NoOps inserted before."""
    import bass_rust
    counter = [0]
    for fn in nc.m.functions:
        for blk in fn.blocks:
            out = []
            changed = False
            for inst in blk.instructions:
                si = inst.sync_info
                limit = _WAIT_LIMIT.get(inst.opcode, 1)
                if (si is not None and len(si.on_wait) > limit
                        and inst.opcode not in _NO_SPLIT_OPCODES):
                    waits = list(si.on_wait)
                    for w in waits[:-limit]:
                        counter[0] += 1
                        nop = bass_rust.InstNoOp(
                            name=f"I-wsplit-{counter[0]}", ins=[], outs=[])
                        nop.engine = inst.engine
                        nop.sync_info = mybir.SyncInfo(on_wait=[w], on_update=[])
                        out.append(nop)
                    inst.sync_info = mybir.SyncInfo(
                        on_wait=waits[-limit:], on_update=list(si.on_update))
                    changed = True
                out.append(inst)
            if changed:
                blk.instructions = out
    return counter[0]


def _build_program():
    nc = bass.Bass()

    ext = {}

    def inp(name, shape, dt):
        ext[name] = nc.declare_dram_parameter(name, list(shape), dt, isOutput=False)
        return ext[name]

    xt4 = inp("xt4", (N_STILE, NC_CHUNKS, P, STILE), BF16)
    wq = inp("wq", (NC_CHUNKS, P, NH_LOC * DK), BF16)
    wk = inp("wk", (NC_CHUNKS, P, DK), BF16)
    wv = inp("wv", (NC_CHUNKS, P, DK), BF16)
    wo = inp("wo", (NH_LOC, P, D), BF16)
    winvq = inp("winvq", (P, 1), BF16)   # 1/(w_q^2 * DK)
    winvk = inp("winvk", (P, 1), BF16)   # 1/w_k^2
    taba = inp("taba", (P, S), BF16)     # [cos; cos]
    tabb = inp("tabb", (P, S), BF16)     # [-sin; sin]
    maskt = inp("maskt", (P, NH_LOC, STILE), BF16)
    bq = inp("bq", (P, NH_LOC), F32)
    bk = inp("bk", (P, 1), F32)
    boeff = inp("boeff", (P, NJB), F32)
    yT = nc.declare_dram_parameter("yT", [D, S], F16, isOutput=True)

    from contextlib import ExitStack

    with tile.TileContext(nc) as tc, ExitStack() as top:
        const = top.enter_context(tc.tile_pool(name="const", bufs=1))

        wq_sb = const.tile([P, NC_CHUNKS, NH_LOC * DK], BF16, tag="wq")
        wk_sb = const.tile([P, NC_CHUNKS, DK], BF16, tag="wk")
        wv_sb = const.tile([P, NC_CHUNKS, DK], BF16, tag="wv")
        wo_sb = const.tile([P, NH_LOC, D], BF16, tag="wo")
        winvq_sb = const.tile([P, 1], BF16, tag="winvq")
        winvk_sb = const.tile([P, 1], BF16, tag="winvk")
        taba_sb = const.tile([P, S], BF16, tag="taba")
        tabb_sb = const.tile([P, S], BF16, tag="tabb")
        mask_sb = const.tile([P, NH_LOC, STILE], BF16, tag="mask")
        bq_sb = const.tile([P, NH_LOC], F32, tag="bq")
        bk_sb = const.tile([P, 1], F32, tag="bk")
        bo_sb = const.tile([P, NJB], F32, tag="bo")

        onesc_b = const.tile([P, 1], BF16, tag="onesc")
        nc.vector.memset(onesc_b[:], 1.0)
        onesr_b = const.tile([1, P], BF16, tag="onesr")
        nc.vector.memset(onesr_b[:], 1.0)
        wrow = const.tile([1, STILE], BF16, tag="wrow")
        nc.vector.memset(wrow[:], 1.0)
        eps_sb = const.tile([P, 1], F32, tag="eps")
        nc.vector.memset(eps_sb[:], EPS)
        epsk_sb = const.tile([P, 1], F32, tag="epsk")
        nc.vector.memset(epsk_sb[:], DK * EPS)

        # persistent activation tensors
        khat = const.tile([P, S], BF16, tag="khat")           # [c, s]
        vsb = const.tile([P, NJB, DK], BF16, tag="v")         # [s%128, block, c]
        scale_k = const.tile([P, NJB], F32, tag="scale_k")    # c0/rms_k, [j%128, block]

        # rotating pools
        xp = top.enter_context(tc.tile_pool(name="xp", bufs=2))
        qhp = top.enter_context(tc.tile_pool(name="qhp", bufs=2))
        aop = top.enter_context(tc.tile_pool(name="aop", bufs=2))
        qfp = top.enter_context(tc.tile_pool(name="qfp", bufs=3))
        sqp = top.enter_context(tc.tile_pool(name="sqp", bufs=3))
        swp = top.enter_context(tc.tile_pool(name="swp", bufs=3))
        t2p = top.enter_context(tc.tile_pool(name="t2p", bufs=4))
        rtp = top.enter_context(tc.tile_pool(name="rtp", bufs=3))
        rowp = top.enter_context(tc.tile_pool(name="rowp", bufs=4))
        rowbp = top.enter_context(tc.tile_pool(name="rowbp", bufs=4))
        pp = top.enter_context(tc.tile_pool(name="pp", bufs=4))
        rlbp = top.enter_context(tc.tile_pool(name="rlbp", bufs=2))
        yp = top.enter_context(tc.tile_pool(name="yp", bufs=3))

        rot = top.enter_context(tc.tile_pool(name="rot", bufs=3, space="PSUM"))
        aops = top.enter_context(tc.tile_pool(name="aops", bufs=2, space="PSUM"))
        plq = top.enter_context(tc.tile_pool(name="plq", bufs=1, space="PSUM"))
        pl2 = top.enter_context(tc.tile_pool(name="pl2", bufs=1, space="PSUM"))
        pskl = top.enter_context(tc.tile_pool(name="pskl", bufs=1, space="PSUM"))

        plq_bank = plq.tile([P, STILE], F32, tag="plq")
        pl2_bank = pl2.tile([P, STILE], F32, tag="pl2")
        ps_kl = pskl.tile([P, NJB], F32, tag="pskl")

        # -------- deferred-emission queue (PE ops needing pipeline cover) ----
        pending = []

        def flush_pending():
            while pending:
                pending.pop(0)()

        # -------------------- startup DMAs --------------------
        # order: what proj(k, st=0) needs first, then v, then q, then attn/out.
        xt0 = xp.tile([P, NC_CHUNKS, STILE], BF16, tag="xt", name="xt0")
        nc.sync.dma_start(xt0[:, 0:4, :], xt4[0, 0:4].rearrange("c p s -> p c s"))
        for g4 in range(0, NC_CHUNKS, 4):
            sl4 = slice(g4, g4 + 4)
            nc.sync.dma_start(wk_sb[:, sl4, :], wk[sl4].rearrange("c p m -> p c m"))
        nc.sync.dma_start(bk_sb[:], bk[:])
        nc.sync.dma_start(winvk_sb[:], winvk[:])
        for g4 in range(0, NC_CHUNKS, 4):
            sl4 = slice(g4, g4 + 4)
            nc.sync.dma_start(wv_sb[:, sl4, :], wv[sl4].rearrange("c p m -> p c m"))
        nc.sync.dma_start(taba_sb[:, 0:STILE], taba[:, 0:STILE])
        nc.sync.dma_start(tabb_sb[:, 0:STILE], tabb[:, 0:STILE])
        for g4 in range(4, NC_CHUNKS, 4):
            sl4 = slice(g4, g4 + 4)
            nc.sync.dma_start(xt0[:, sl4, :], xt4[0, sl4].rearrange("c p s -> p c s"))
        for g4 in range(0, NC_CHUNKS, 4):
            sl4 = slice(g4, g4 + 4)
            nc.sync.dma_start(wq_sb[:, sl4, :], wq[sl4].rearrange("c p m -> p c m"))
        nc.sync.dma_start(bq_sb[:], bq[:])
        nc.sync.dma_start(winvq_sb[:], winvq[:])
        for st_ in range(1, N_STILE):
            ssl_ = bass.ts(st_, STILE)
            nc.sync.dma_start(taba_sb[:, ssl_], taba[:, ssl_])
            nc.sync.dma_start(tabb_sb[:, ssl_], tabb[:, ssl_])

        # -------------------- PE warm-up (HAM clock gate) --------------------
        for w_ in range(N_WARM):
            wt = rot.tile([P, STILE], F32, tag="rot", name=f"warm{w_}")
            nc.tensor.matmul(wt[:], onesr_b[:], wrow[:], start=True, stop=True)

        # state carried across pipeline stages
        qhat_t = {}   # st -> tile [P, NH_LOC, STILE]
        aon_t = {}    # st -> tile [P, NH_LOC, STILE]
        xt_t = {0: xt0}

        # ==================== section emitters ====================

        def emit_proj_mm(st, oi, xt):
            """PE projection matmuls for one output (q-head / 'k' / 'v')."""
            if oi == "v":
                ptv = rot.tile([P, STILE], F32, tag="rot", name=f"ptv{st}")
                for sb in range(4):
                    for ch in range(NC_CHUNKS):
                        nc.tensor.matmul(ptv[:, bass.ts(sb, DK)],
                                         xt[:, ch, bass.ts(sb, P)], wv_sb[:, ch, :],
                                         start=(ch == 0), stop=(ch == NC_CHUNKS - 1))
                return ptv
            pt = rot.tile([P, STILE], F32, tag="rot", name=f"pt{st}{oi}")
            for ch in range(NC_CHUNKS):
                lw = wk_sb[:, ch, :] if oi == "k" else wq_sb[:, ch, bass.ts(oi, DK)]
                nc.tensor.matmul(pt[:], lw, xt[:, ch, :],
                                 start=(ch == 0), stop=(ch == NC_CHUNKS - 1))
            return pt

        def emit_tail1(st, oi, pt):
            """Off-PE tail right after proj matmuls: bias+bf16 (ACT), sumsq,
            RoPE (DVE). Returns handles used by the deferred PE tail."""
            ssl = bass.ts(st, STILE)
            if oi == "v":
                nc.vector.tensor_copy(vsb[:, st * 4:(st + 1) * 4, :], pt[:])
                return None
            is_q = oi != "k"
            bias_ap = bq_sb[:, oi:oi + 1] if is_q else bk_sb[:, 0:1]
            qf = qfp.tile([P, STILE], BF16, tag="qf")
            nc.scalar.activation(qf[:], pt[:], AF.Identity, bias=bias_ap)
            sq = sqp.tile([P, STILE], BF16, tag="sq")
            nc.vector.tensor_mul(sq[:], qf[:], qf[:])
            # RoPE on de-interleaved halves:
            #   rt = qf*[cos;cos] + swap_halves(qf)*[-sin;sin]
            sw = swp.tile([P, STILE], BF16, tag="sw")
            nc.sync.dma_start(sw[0:64, :], qf[64:P, :])
            nc.sync.dma_start(sw[64:P, :], qf[0:64, :])
            ta = t2p.tile([P, STILE], BF16, tag="ta")
            tb = t2p.tile([P, STILE], BF16, tag="tb")
            nc.vector.tensor_mul(ta[:], qf[:], taba_sb[:, ssl])
            nc.vector.tensor_mul(tb[:], sw[:], tabb_sb[:, ssl])
            if is_q:
                rt = rtp.tile([P, STILE], BF16, tag="rt")
                nc.vector.tensor_add(rt[:], ta[:], tb[:])
                return sq, rt
            nc.vector.tensor_add(khat[:, ssl], ta[:], tb[:])
            return sq, None

        def emit_pe_tail1(st, oi, sq):
            """PE sumsq for output oi (emitted one proj-group later)."""
            if oi == "k":
                for sb in range(4):
                    col = st * 4 + sb
                    nc.tensor.matmul(ps_kl[:, col:col + 1],
                                     sq[:, bass.ts(sb, P)], winvk_sb[:],
                                     start=True, stop=True)
                # scale_k = C0/rms_k = 1/sqrt(DK*ms + DK*eps) = exp(-ln(.)/2)
                ksl4 = bass.ts(st, 4)
                skf = rowp.tile([P, 4], F32, tag="skf")
                nc.scalar.activation(skf[:], ps_kl[:, ksl4], AF.Ln,
                                     bias=epsk_sb[:], scale=1.0)
                nc.scalar.activation(scale_k[:, ksl4], skf[:], AF.Exp, scale=-0.5)
            else:
                h = oi
                out = plq_bank[32 * h:32 * h + 1, :]
                nc.tensor.matmul(out, winvq_sb[:], sq[:], start=True, stop=True,
                                 tile_position=(0, 32 * h))
                # rrq = 1/rms_q = 1/sqrt(ms + eps) = exp(-ln(ms + eps)/2)
                rrq_f = rowp.tile([1, STILE], F32, tag="rrqf")
                nc.scalar.activation(rrq_f[:], plq_bank[32 * h:32 * h + 1, :],
                                     AF.Ln, bias=eps_sb[0:1, :], scale=1.0)
                rrq_b = rowbp.tile([1, STILE], BF16, tag="rrqb")
                nc.scalar.activation(rrq_b[:], rrq_f[:], AF.Exp, scale=-0.5)
                return rrq_b

        def emit_pe_tail2(st, h, rt, rrq_b):
            """PE broadcast of 1/rms_q + final qhat mul (two groups later)."""
            pb = rot.tile([P, STILE], F32, tag="rot", name=f"pb{st}{h}")
            nc.tensor.matmul(pb[:], onesr_b[:], rrq_b[:], start=True, stop=True)
            nc.vector.tensor_mul(qhat_t[st][:, h, :], rt[:], pb[:])

        def emit_proj(st):
            xt = xt_t[st]
            qhat_t[st] = qhp.tile([P, NH_LOC, STILE], BF16, tag="qhat", name=f"qhat{st}")

            seq = ["k", "v", 0, 1, 2, 3]
            tails = {}
            for idx, oi in enumerate(seq):
                pt = emit_proj_mm(st, oi, xt)
                if idx >= 1:
                    flush_pending()
                tails[oi] = emit_tail1(st, oi, pt)
                prev = seq[idx - 1] if idx >= 1 else None
                if prev is not None and tails[prev] is not None:
                    sq_p, rt_p = tails[prev]
                    rrq_b = emit_pe_tail1(st, prev, sq_p)
                    if rt_p is not None:
                        pending.append(
                            lambda st=st, h=prev, rt=rt_p, rb=rrq_b:
                            emit_pe_tail2(st, h, rt, rb))
            # tail of the last output (q3): sumsq now, broadcast deferred
            sq_p, rt_p = tails[3]
            rrq_b = emit_pe_tail1(st, 3, sq_p)
            pending.append(
                lambda st=st, rt=rt_p, rb=rrq_b: emit_pe_tail2(st, 3, rt, rb))
            # prefetch next x tile; late-stage weights after the first prefetch
            if st + 1 < N_STILE:
                xt_n = xp.tile([P, NC_CHUNKS, STILE], BF16, tag="xt",
                               name=f"xt{st + 1}")
                for g4 in range(0, NC_CHUNKS, 4):
                    sl4 = slice(g4, g4 + 4)
                    nc.sync.dma_start(xt_n[:, sl4, :],
                                      xt4[st + 1, sl4].rearrange("c p s -> p c s"))
                xt_t[st + 1] = xt_n
            if st == 0:
                nc.sync.dma_start(mask_sb[:], maskt[:])
                nc.sync.dma_start(bo_sb[:], boeff[:])
            elif st == 1:
                nc.sync.dma_start(wo_sb[:], wo.rearrange("f p e -> p f e"))

        def emit_attn_tile_end(st, h, ao_ps):
            """softmax-normalize: 1/l, broadcast on PE, multiply into aon."""
            rl_f = rowp.tile([1, STILE], F32, tag="rlf")
            nc.scalar.activation(rl_f[:], pl2_bank[32 * h:32 * h + 1, :], AF.Ln)
            rl_b = rowbp.tile([1, STILE], BF16, tag="rlb")
            nc.scalar.activation(rl_b[:], rl_f[:], AF.Exp, scale=-1.0)

            def bcast(st=st, h=h, ao_ps=ao_ps, rl_b=rl_b):
                rlb = rot.tile([P, STILE], F32, tag="rot", name=f"rlb{st}{h}")
                nc.tensor.matmul(rlb[:], onesr_b[:], rl_b[:], start=True, stop=True)
                rlb_sb = rlbp.tile([P, STILE], BF16, tag="rlbs")
                nc.vector.tensor_copy(rlb_sb[:], rlb[:])
                nc.vector.tensor_mul(aon_t[st][:, h, :], ao_ps[:], rlb_sb[:])
            pending.append(bcast)

        def emit_attn(it):
            aon_t[it] = aop.tile([P, NH_LOC, STILE], BF16, tag="aon", name=f"aon{it}")
            njb = 4 * it + 4
            SKEW = 2
            for h in range(NH_LOC):
                ao_ps = aops.tile([P, STILE], F32, tag="ao")
                pts = {}
                colsl = {}
                for x in range(njb + SKEW):
                    if x < njb:
                        jb = x
                        t = jb - 4 * it
                        lo = P * t if t > 0 else 0
                        csl = slice(lo, STILE)
                        colsl[jb] = (csl, t)
                        i0 = it * STILE + lo
                        st_ps = rot.tile([P, STILE], F32, tag="rot",
                                         name=f"st{it}{h}{jb}")
                        nc.tensor.matmul(st_ps[:, csl], khat[:, bass.ts(jb, P)],
                                         qhat_t[it][:, h, bass.ds(lo, STILE - lo)],
                                         start=True, stop=True)
                        pt8 = pp.tile([P, STILE], BF16, tag="p")
                        nc.scalar.activation(pt8[:, csl], st_ps[:, csl], AF.Exp,
                                             scale=scale_k[:, jb:jb + 1])
                        if t >= 0:
                            nc.vector.tensor_mul(pt8[:, csl], pt8[:, csl],
                                                 mask_sb[:, t, csl])
                        pts[jb] = pt8
                        if x == 1:
                            flush_pending()
                    if x >= SKEW:
                        jb = x - SKEW
                        csl, t = colsl[jb]
                        pt8 = pts.pop(jb)
                        nc.tensor.matmul(ao_ps[:, csl], vsb[:, jb, :], pt8[:, csl],
                                         start=(jb == 0), stop=(jb == njb - 1))
                        nc.tensor.matmul(pl2_bank[32 * h:32 * h + 1, csl],
                                         onesc_b[:], pt8[:, csl],
                                         start=(jb == 0), stop=(jb == njb - 1),
                                         tile_position=(0, 32 * h))
                emit_attn_tile_end(it, h, ao_ps)

        def emit_outproj(st):
            yT_v = yT.rearrange("(eb p) s -> eb p s", p=P)
            for eb in range(NJB):
                yps = rot.tile([P, STILE], F32, tag="rot", name=f"y{st}{eb}")
                for fc in range(NH_LOC):
                    nc.tensor.matmul(yps[:], wo_sb[:, fc, bass.ts(eb, P)],
                                     aon_t[st][:, fc, :],
                                     start=(fc == 0), stop=(fc == NH_LOC - 1))
                if eb % 4 == 1:
                    flush_pending()
                y_sb = yp.tile([P, STILE], F16, tag="ysb")
                nc.vector.tensor_scalar_add(y_sb[:], yps[:], bo_sb[:, eb:eb + 1])
                nc.sync.dma_start(yT_v[eb, :, bass.ts(st, STILE)], y_sb[:])

        # ==================== main pipelined loop ====================
        for st in range(N_STILE):
            with nc.named_scope(f"proj{st}"):
                emit_proj(st)
            if st >= 1:
                with nc.named_scope(f"attn{st - 1}"):
                    emit_attn(st - 1)
            if st >= 2:
                with nc.named_scope(f"out{st - 2}"):
                    emit_outproj(st - 2)
        with nc.named_scope("attn3"):
            emit_attn(3)
        with nc.named_scope("out2"):
            emit_outproj(2)
        with nc.named_scope("out3"):
            emit_outproj(3)
        flush_pending()

    _split_excess_waits(nc)
    return nc


_PERM = np.concatenate([np.arange(0, DK, 2), np.arange(1, DK, 2)])  # de-interleave


def _prep_inputs(x, Wq, bq, Wk, bk, Wv, bv, Wo, bo, q_norm_w, k_norm_w):
    """Build the 8 per-core input maps. Core c -> (b = c // 4, g = c % 4)."""
    def bf(a):
        return np.ascontiguousarray(a).astype(_BF)

    wq_p = q_norm_w[_PERM].astype(np.float32)
    wk_p = k_norm_w[_PERM].astype(np.float32)
    with np.errstate(divide="ignore"):
        winvq = np.where(wq_p != 0,
                         1.0 / np.maximum(wq_p * wq_p * DK, 1e-30), 0.0)
        winvk = np.where(wk_p != 0, 1.0 / np.maximum(wk_p * wk_p, 1e-30), 0.0)

    inv_freq = 1.0 / (10000.0 ** (np.arange(0, DK, 2, dtype=np.float32) / np.float32(DK)))
    freqs = np.arange(S, dtype=np.float32)[:, None] * inv_freq[None, :]
    cosT = np.cos(freqs).T.astype(np.float32)  # [64, S]
    sinT = np.sin(freqs).T.astype(np.float32)
    taba = np.vstack([cosT, cosT]).copy()          # [128, S]
    tabb = np.vstack([-sinT, sinT]).copy()

    pj = np.arange(P)[:, None, None]
    tt = np.arange(NH_LOC)[None, :, None]
    fi = np.arange(STILE)[None, None, :]
    maskt = ((P * tt + pj) <= fi).astype(np.float32)  # [128, 4, 512]

    xt4_b = []
    for b in range(2):
        xt = x[b].T.astype(np.float32)  # [d, s]
        xt4_b.append(bf(xt.reshape(NC_CHUNKS, P, N_STILE, STILE).transpose(2, 0, 1, 3)))

    in_maps = []
    for core in range(8):
        b, g = divmod(core, NH_LOC)
        hsl = slice(g * NH_LOC * DK, (g + 1) * NH_LOC * DK)
        ksl = slice(g * DK, (g + 1) * DK)

        wq_blk = Wq[hsl].astype(np.float32).copy()  # [512, d]
        # per-head de-interleave permutation + fold q_norm_w
        wq_blk = wq_blk.reshape(NH_LOC, DK, D)[:, _PERM, :] * wq_p[None, :, None]
        wq_t = wq_blk.reshape(NH_LOC * DK, D).T.reshape(NC_CHUNKS, P, NH_LOC * DK)

        wk_blk = Wk[ksl].astype(np.float32)[_PERM, :] * wk_p[:, None]
        wk_t = wk_blk.T.reshape(NC_CHUNKS, P, DK)
        wv_t = Wv[ksl].astype(np.float32).T.reshape(NC_CHUNKS, P, DK)
        wo_t = Wo[:, hsl].astype(np.float32).T.reshape(NH_LOC, P, D)

        # fold the norm weight into the bias too (wq has w folded in)
        bq_blk = (bq[hsl].astype(np.float32).reshape(NH_LOC, DK)[:, _PERM]
                  * wq_p[None, :]).T.copy()  # [128, 4]
        bk_blk = (bk[ksl].astype(np.float32)[_PERM] * wk_p)[:, None].copy()
        if g == 0:
            boeff = bo.astype(np.float32).reshape(NJB, P).T.copy()
        else:
            boeff = np.zeros((P, NJB), np.float32)

        in_maps.append({
            "xt4": xt4_b[b],
            "wq": bf(wq_t), "wk": bf(wk_t), "wv": bf(wv_t), "wo": bf(wo_t),
            "winvq": bf(winvq[:, None]), "winvk": bf(winvk[:, None]),
            "taba": bf(taba), "tabb": bf(tabb),
            "maskt": bf(maskt),
            "bq": np.ascontiguousarray(bq_blk), "bk": bk_blk, "boeff": boeff,
        })
    return in_maps


_CACHED = {}


def _get_program():
    if "nc" not in _CACHED:
        _CACHED["nc"] = _build_program()
    return _CACHED["nc"]


def kernel(x, Wq, bq, Wk, bk, Wv, bv, Wo, bo, q_norm_w, k_norm_w, _trace=False, _tmpdir=None):
    x = np.asarray(x, np.float32)
    args = [np.asarray(a, np.float32) for a in
            (Wq, bq, Wk, bk, Wv, bv, Wo, bo, q_norm_w, k_norm_w)]
    Wq, bq, Wk, bk, Wv, bv, Wo, bo, q_norm_w, k_norm_w = args

    nc = _get_program()
    in_maps = _prep_inputs(x, Wq, bq, Wk, bk, Wv, bv, Wo, bo, q_norm_w, k_norm_w)
    res = run_bass_kernel_spmd(nc, in_maps, list(range(8)), trace=_trace, tmpdir=_tmpdir)

    out = np.zeros((2, S, D), np.float32)
    for core in range(8):
        b = core // 4
        out[b] += res.results[core]["yT"].astype(np.float32).T
    # v-bias enters only via softmax-weighted average (weights sum to 1):
    if np.any(bv):
        out += (np.repeat(bv.reshape(4, DK), 4, axis=0).reshape(D) @ Wo.T)[None, None, :]
    kernel._last_result = res
    return out
